# revision 8
# baseline (speedup 1.0000x reference)
"""AttnMatch Trainium2 kernel: Bahdanau-style additive attention.

  rs       = rnn_state.reshape(B, D)
  att_proj = relu(att_hidden @ W.T + (rs @ U.T)[:, None, :])   # [B,S,D]
  scores   = einsum('bse,be->bs', att_proj, rs)                # [B,S]
  soft     = softmax(scores, axis=1)                           # [B,S]
  matched  = einsum('bsd,bs->bd', att_hidden, soft)            # [B,D]
  returns (matched, soft)

Sharding: data-parallel over batch B=32 across 8 NeuronCores (4 batches/core).
Each core runs the identical program on its own batch slice; no collectives.

Compute dtype: float32r (TF32-labelled, but full fp32-precision on TRN2 HW per
measurement) for all TensorE matmuls; fp32 accumulation in PSUM.

Per-core, per-batch plan (S=2048, D=1024):
  mm1  P.T[e,s] += W.T[d,e].T @ A.T[d,s]  (8 K-chunks of 128, N=512 tiles)
  ACT  pt = relu(psum + u[e])             (u[e] = per-partition bias)
  mm2  scores[4,s] += rsT[e,1:4].T @ pt   (rs stationary, N=512)
  softmax on scores row b ([1,2048] at partition b)
  PE-transpose soft row -> [128,16] partition-major chunks
  mm3  matched[1,d] += soft_chunk.T @ A_nat[s,d]  (16 K-chunks, N=512)
"""

import numpy as np

import concourse.bass as bass
import concourse.mybir as mybir
import concourse.tile as tile
from concourse import bacc, bass_utils
from concourse.masks import make_identity

B, S, D = 32, 2048, 1024
NCORES = 8
BPC = B // NCORES          # batches per core
DC = D // 128              # contraction chunks of 128
SI = S // 512              # s-tiles of 512
F32 = mybir.dt.float32

_CACHE = {}


def build_program(mm_dt=mybir.dt.float32r):
    nc = bacc.Bacc("TRN2", target_bir_lowering=False, debug=False,
                   num_devices=NCORES)
    at_d = nc.dram_tensor("at", [BPC, D, S], mm_dt, kind="ExternalInput").ap()
    an_d = nc.dram_tensor("an", [BPC, S, D], mm_dt, kind="ExternalInput").ap()
    wt_d = nc.dram_tensor("wt", [D, D], mm_dt, kind="ExternalInput").ap()
    ut_d = nc.dram_tensor("ut", [D, D], mm_dt, kind="ExternalInput").ap()
    rst_d = nc.dram_tensor("rst", [D, BPC], mm_dt, kind="ExternalInput").ap()
    matched_d = nc.dram_tensor("matched", [BPC, D], F32, kind="ExternalOutput").ap()
    soft_d = nc.dram_tensor("soft", [BPC, S], F32, kind="ExternalOutput").ap()

    Relu = mybir.ActivationFunctionType.Relu
    Exp = mybir.ActivationFunctionType.Exp

    with tile.TileContext(nc) as tc:
        with (
            tc.tile_pool(name="const", bufs=1) as const_pool,
            tc.tile_pool(name="sbuf", bufs=3) as pool,
            tc.tile_pool(name="psum", bufs=2, space="PSUM") as psum_pool,
        ):
            # ---------------- constants / preamble ----------------
            identity = const_pool.tile([128, 128], F32, tag="ident")
            make_identity(nc, identity)

            # W.T resident: [d,e] -> sbuf [128, DC, D]
            wt_sb = const_pool.tile([128, DC, D], mm_dt, tag="wt")
            nc.sync.dma_start(out=wt_sb, in_=wt_d.rearrange("(c p) e -> p c e", p=128))

            # rs.T resident: [d,b] -> sbuf [128, DC, BPC]
            rst_sb = const_pool.tile([128, DC, BPC], mm_dt, tag="rst")
            nc.sync.dma_start(out=rst_sb, in_=rst_d.rearrange("(c p) b -> p c b", p=128))

            # u[b,e] = U @ rs_b : computed as  rsT.T @ U.T  -> [BPC, D]
            ut_tiles = []
            for dc in range(DC):
                t = pool.tile([128, D], mm_dt, tag="anat", bufs=18)
                nc.sync.dma_start(
                    out=t, in_=ut_d.rearrange("(c p) e -> p c e", p=128)[:, dc, :])
                ut_tiles.append(t)
            u_ps = []
            for h in range(2):
                ps = psum_pool.tile([BPC, 512], F32, tag="sc4")
                for dc in range(DC):
                    nc.tensor.matmul(ps, lhsT=rst_sb[:, dc, :],
                                     rhs=ut_tiles[dc][:, h * 512:(h + 1) * 512],
                                     start=(dc == 0), stop=(dc == DC - 1))
                u_ps.append(ps)
            u_be = const_pool.tile([BPC, D], F32, tag="u_be")
            nc.scalar.copy(u_be[:, :512], u_ps[0])
            nc.scalar.copy(u_be[:, 512:], u_ps[1])
            # transpose u to [e_part, ec, b] for use as per-partition bias
            uT_sb = const_pool.tile([128, DC, BPC], F32, tag="uT")
            for ec in range(DC):
                ps_tr = psum_pool.tile([128, BPC], F32, tag="tr")
                nc.tensor.transpose(ps_tr, u_be[0:BPC, ec * 128:(ec + 1) * 128],
                                    identity[0:BPC, 0:BPC])
                nc.scalar.copy(uT_sb[:, ec, :], ps_tr)


            # ---------------- main loop over batches ----------------
            for b in range(BPC):
                # prefetch A natural-layout chunks for mm3 (consumed late)
                anat_tiles = []
                for i in range(S // 128):
                    t = pool.tile([128, D], mm_dt, tag="anat", bufs=18)
                    nc.sync.dma_start(out=t, in_=an_d[b, i * 128:(i + 1) * 128, :])
                    anat_tiles.append(t)

                # per-batch scores tile; only row b is meaningful (the mm2
                # stationary operand is all 4 batches' rs, so rows != b hold
                # finite-but-unused cross-batch products)
                soft_sb = pool.tile([BPC, S], F32, tag="scores", bufs=2)

                for si in range(SI):
                    at_tile = pool.tile([128, DC, 512], mm_dt, tag="at", bufs=3)
                    nc.sync.dma_start(
                        out=at_tile,
                        in_=at_d[b].rearrange("(c p) s -> p c s", p=128)
                        [:, :, si * 512:(si + 1) * 512])
                    psum_sc = psum_pool.tile([BPC, 512], F32, tag="sc4")
                    for e in range(DC):
                        psum_p = psum_pool.tile([128, 512], F32, tag="big")
                        for dc in range(DC):
                            nc.tensor.matmul(
                                psum_p,
                                lhsT=wt_sb[:, dc, e * 128:(e + 1) * 128],
                                rhs=at_tile[:, dc, :],
                                start=(dc == 0), stop=(dc == DC - 1))
                        pt_tile = pool.tile([128, 512], mm_dt, tag="pt", bufs=4)
                        nc.scalar.activation(pt_tile, psum_p, Relu,
                                             bias=uT_sb[:, e, b:b + 1], scale=1.0)
                        nc.tensor.matmul(psum_sc, lhsT=rst_sb[:, e, :],
                                         rhs=pt_tile,
                                         start=(e == 0), stop=(e == DC - 1))
                    nc.scalar.copy(soft_sb[:, si * 512:(si + 1) * 512], psum_sc)

                # softmax: run on all 4 rows (partition-parallel, same cost);
                # only row b is meaningful. Engine APs must start at
                # partition 0, so never slice rows on compute ops.
                negm_sb = pool.tile([BPC, 1], F32, tag="negm", bufs=2)
                sumexp_sb = pool.tile([BPC, 1], F32, tag="sumexp", bufs=2)
                rz_sb = pool.tile([BPC, 1], F32, tag="rz", bufs=2)
                nc.vector.reduce_max(negm_sb, soft_sb,
                                     axis=mybir.AxisListType.X, negate=True)
                nc.scalar.activation(soft_sb, soft_sb,
                                     Exp, bias=negm_sb, scale=1.0,
                                     accum_out=sumexp_sb)
                nc.vector.reciprocal(rz_sb, sumexp_sb)
                nc.vector.tensor_scalar_mul(soft_sb, soft_sb, rz_sb)
                nc.sync.dma_start(out=soft_d[b:b + 1, :], in_=soft_sb[b:b + 1, :])

                # transpose soft row -> [128, S//128] partition-major
                psum_tr = psum_pool.tile([128, (S // 128) * BPC], F32, tag="tr")
                for i in range(S // 128):
                    nc.tensor.transpose(psum_tr[:, i * BPC:(i + 1) * BPC],
                                        soft_sb[0:BPC, i * 128:(i + 1) * 128],
                                        identity[0:BPC, 0:BPC])
                soft_pt = pool.tile([128, (S // 128) * BPC], mm_dt, tag="softpt", bufs=2)
                nc.scalar.copy(soft_pt, psum_tr)

                # mm3: matched[d] = sum_s soft[s] * A[s, d]
                psum_m = [psum_pool.tile([1, 512], F32, tag="m", name=f"psum_m{j}")
                          for j in range(2)]
                for i in range(S // 128):
                    for j in range(2):
                        nc.tensor.matmul(
                            psum_m[j],
                            lhsT=soft_pt[:, i * BPC + b:i * BPC + b + 1],
                            rhs=anat_tiles[i][:, j * 512:(j + 1) * 512],
                            start=(i == 0), stop=(i == S // 128 - 1))
                matched_sb = pool.tile([1, D], F32, tag="matched", bufs=2)
                nc.scalar.copy(matched_sb[:, :512], psum_m[0])
                nc.scalar.copy(matched_sb[:, 512:], psum_m[1])
                nc.sync.dma_start(out=matched_d[b:b + 1, :], in_=matched_sb)

    nc.compile()
    return nc


def prep_inputs(att_hidden, rnn_state, W, U):
    rs = np.ascontiguousarray(rnn_state.reshape(B, D), dtype=np.float32)
    at = np.ascontiguousarray(att_hidden.transpose(0, 2, 1), dtype=np.float32)
    an = np.ascontiguousarray(att_hidden, dtype=np.float32)
    wt = np.ascontiguousarray(W.T, dtype=np.float32)
    ut = np.ascontiguousarray(U.T, dtype=np.float32)
    rst = np.ascontiguousarray(rs.T, dtype=np.float32)
    in_maps = []
    for c in range(NCORES):
        sl = slice(c * BPC, (c + 1) * BPC)
        in_maps.append({
            "at": at[sl], "an": an[sl], "wt": wt, "ut": ut,
            "rst": np.ascontiguousarray(rst[:, sl]),
        })
    return in_maps


def assemble(results):
    matched = np.concatenate([results[c]["matched"] for c in range(NCORES)], axis=0)
    soft = np.concatenate([results[c]["soft"] for c in range(NCORES)], axis=0)
    return matched.astype(np.float32), soft.astype(np.float32)


def kernel(att_hidden, rnn_state, W, U):
    if "nc" not in _CACHE:
        _CACHE["nc"] = build_program()
    in_maps = prep_inputs(att_hidden, rnn_state, W, U)
    res = bass_utils.run_bass_kernel_spmd(
        _CACHE["nc"], in_maps, core_ids=list(range(NCORES)))
    return assemble(res.results)


# revision 9
# speedup vs baseline: 1.0635x; 1.0635x over previous
"""AttnMatch Trainium2 kernel: Bahdanau-style additive attention.

  rs       = rnn_state.reshape(B, D)
  att_proj = relu(att_hidden @ W.T + (rs @ U.T)[:, None, :])   # [B,S,D]
  scores   = einsum('bse,be->bs', att_proj, rs)                # [B,S]
  soft     = softmax(scores, axis=1)                           # [B,S]
  matched  = einsum('bsd,bs->bd', att_hidden, soft)            # [B,D]
  returns (matched, soft)

Sharding: data-parallel over batch B=32 across 8 NeuronCores (4 batches/core).
Each core runs the identical program on its own batch slice; no collectives.

Compute dtype: float32r for all TensorE matmuls (full fp32 precision on TRN2
HW -- single fp32_mode=HIGH pass -- at bf16-class streaming rate); fp32
accumulation in PSUM.

Per-core, per-batch plan (S=2048, D=1024):
  mm1  P.T[e,s] += W.T[d,e].T @ A.T[d,s]  (8 K-chunks of 128, N=512 tiles)
  ACT  pt = relu(psum + u[e])             (u[e] = per-partition bias)
  mm2  scores[4,s] += rsT[e,0:4].T @ pt   (rs stationary, N=512)
  softmax on all 4 rows at partition base 0 (row b meaningful)
  PE-transpose soft row -> [128,16] partition-major chunks
  mm3  matched[1,d] += soft_chunk.T @ A_nat[s,d]  (16 K-chunks, N=512)

DMA choreography: W.T/A.T/U.T loads are split per-128-chunk so they spread
across DMA queues; A-natural (mm3) prefetch is interleaved after each s-tile
so it never starves the mm1-critical A.T loads.
"""

import numpy as np

import concourse.bass as bass
import concourse.mybir as mybir
import concourse.tile as tile
from concourse import bacc, bass_utils
from concourse.masks import make_identity

B, S, D = 32, 2048, 1024
NCORES = 8
BPC = B // NCORES          # batches per core
DC = D // 128              # contraction chunks of 128
SI = S // 512              # s-tiles of 512
SC = S // 128              # s-chunks of 128
F32 = mybir.dt.float32

_CACHE = {}


def build_program(mm_dt=mybir.dt.float32r):
    nc = bacc.Bacc("TRN2", target_bir_lowering=False, debug=False,
                   num_devices=NCORES)
    at_d = nc.dram_tensor("at", [BPC, D, S], mm_dt, kind="ExternalInput").ap()
    an_d = nc.dram_tensor("an", [BPC, S, D], mm_dt, kind="ExternalInput").ap()
    wt_d = nc.dram_tensor("wt", [D, D], mm_dt, kind="ExternalInput").ap()
    ut_d = nc.dram_tensor("ut", [D, D], mm_dt, kind="ExternalInput").ap()
    rst_d = nc.dram_tensor("rst", [D, BPC], mm_dt, kind="ExternalInput").ap()
    matched_d = nc.dram_tensor("matched", [BPC, D], F32, kind="ExternalOutput").ap()
    soft_d = nc.dram_tensor("soft", [BPC, S], F32, kind="ExternalOutput").ap()

    Relu = mybir.ActivationFunctionType.Relu
    Exp = mybir.ActivationFunctionType.Exp

    with tile.TileContext(nc) as tc:
        with (
            tc.tile_pool(name="const", bufs=1) as const_pool,
            tc.tile_pool(name="sbuf", bufs=3) as pool,
            tc.tile_pool(name="psum", bufs=2, space="PSUM") as psum_pool,
        ):
            # ---------------- constants ----------------
            identity = const_pool.tile([128, 128], F32, tag="ident")
            make_identity(nc, identity)

            # W.T resident: [d,e] -> sbuf [128, DC, D]; per-chunk DMAs so the
            # 4MB load spreads over queues and finishes fast
            wt_view = wt_d.rearrange("(c p) e -> p c e", p=128)
            wt_sb = const_pool.tile([128, DC, D], mm_dt, tag="wt")
            for dc in range(DC):
                nc.sync.dma_start(out=wt_sb[:, dc, :], in_=wt_view[:, dc, :])

            # rs.T resident: [d,b] -> sbuf [128, DC, BPC]
            rst_sb = const_pool.tile([128, DC, BPC], mm_dt, tag="rst")
            nc.sync.dma_start(out=rst_sb, in_=rst_d.rearrange("(c p) b -> p c b", p=128))

            uT_sb = const_pool.tile([128, DC, BPC], F32, tag="uT")

            def emit_u_preamble():
                # u[b,e] = U @ rs_b, computed as rsT.T @ U.T -> [BPC, D],
                # then PE-transposed into [e_part, ec, b] bias layout
                ut_tiles = []
                for dc in range(DC):
                    t = pool.tile([128, D], mm_dt, tag="anat", bufs=18)
                    nc.sync.dma_start(
                        out=t, in_=ut_d.rearrange("(c p) e -> p c e", p=128)[:, dc, :])
                    ut_tiles.append(t)
                u_ps = []
                for h in range(2):
                    ps = psum_pool.tile([BPC, 512], F32, tag="sc4",
                                        name=f"u_ps{h}")
                    for dc in range(DC):
                        nc.tensor.matmul(ps, lhsT=rst_sb[:, dc, :],
                                         rhs=ut_tiles[dc][:, h * 512:(h + 1) * 512],
                                         start=(dc == 0), stop=(dc == DC - 1))
                    u_ps.append(ps)
                u_be = const_pool.tile([BPC, D], F32, tag="u_be")
                nc.scalar.copy(u_be[:, :512], u_ps[0])
                nc.scalar.copy(u_be[:, 512:], u_ps[1])
                for ec in range(DC):
                    ps_tr = psum_pool.tile([128, BPC], F32, tag="tr", bufs=1)
                    nc.tensor.transpose(ps_tr, u_be[0:BPC, ec * 128:(ec + 1) * 128],
                                        identity[0:BPC, 0:BPC])
                    nc.scalar.copy(uT_sb[:, ec, :], ps_tr)

            # ---------------- main loop over batches ----------------
            for b in range(BPC):
                # per-batch scores tile; only row b is meaningful (the mm2
                # stationary operand is all 4 batches' rs, so rows != b hold
                # finite-but-unused cross-batch products)
                soft_sb = pool.tile([BPC, S], F32, tag="scores", bufs=2)
                anat_tiles = [None] * SC

                at_view = at_d[b].rearrange("(c p) s -> p c s", p=128)
                for si in range(SI):
                    # A.T chunks for this s-tile, one DMA per 128-d-chunk
                    at_tiles = []
                    for dc in range(DC):
                        t = pool.tile([128, 512], mm_dt, tag="at", bufs=24,
                                      name=f"at_{dc}")
                        nc.sync.dma_start(
                            out=t, in_=at_view[:, dc, si * 512:(si + 1) * 512])
                        at_tiles.append(t)

                    if b == 0 and si == 0:
                        # u preamble rides alongside the first s-tile: its DMAs
                        # queue after the mm1-critical W.T/A.T loads, and the
                        # first relu-bias consumer is ~3 matmul groups in
                        emit_u_preamble()

                    psum_sc = psum_pool.tile([BPC, 512], F32, tag="sc4")
                    for e in range(DC):
                        psum_p = psum_pool.tile([128, 512], F32, tag="big", bufs=3)
                        for dc in range(DC):
                            nc.tensor.matmul(
                                psum_p,
                                lhsT=wt_sb[:, dc, e * 128:(e + 1) * 128],
                                rhs=at_tiles[dc],
                                start=(dc == 0), stop=(dc == DC - 1))
                        pt_tile = pool.tile([128, 512], mm_dt, tag="pt", bufs=4)
                        nc.scalar.activation(pt_tile, psum_p, Relu,
                                             bias=uT_sb[:, e, b:b + 1], scale=1.0)
                        nc.tensor.matmul(psum_sc, lhsT=rst_sb[:, e, :],
                                         rhs=pt_tile,
                                         start=(e == 0), stop=(e == DC - 1))
                    nc.scalar.copy(soft_sb[:, si * 512:(si + 1) * 512], psum_sc)

                    # interleave mm3 prefetch: 4 A-natural chunks per s-tile
                    for i in range(si * 4, si * 4 + 4):
                        t = pool.tile([128, D], mm_dt, tag="anat", bufs=18,
                                      name=f"anat_{i}")
                        nc.sync.dma_start(out=t, in_=an_d[b, i * 128:(i + 1) * 128, :])
                        anat_tiles[i] = t

                # softmax: run on all 4 rows (partition-parallel, same cost);
                # only row b is meaningful. Engine APs must start at
                # partition 0, so never slice rows on compute ops.
                negm_sb = pool.tile([BPC, 1], F32, tag="negm", bufs=2)
                sumexp_sb = pool.tile([BPC, 1], F32, tag="sumexp", bufs=2)
                rz_sb = pool.tile([BPC, 1], F32, tag="rz", bufs=2)
                nc.vector.reduce_max(negm_sb, soft_sb,
                                     axis=mybir.AxisListType.X, negate=True)
                nc.scalar.activation(soft_sb, soft_sb,
                                     Exp, bias=negm_sb, scale=1.0,
                                     accum_out=sumexp_sb)
                nc.vector.reciprocal(rz_sb, sumexp_sb)
                nc.vector.tensor_scalar_mul(soft_sb, soft_sb, rz_sb)
                nc.sync.dma_start(out=soft_d[b:b + 1, :], in_=soft_sb[b:b + 1, :])

                # transpose soft row -> [128, SC] partition-major
                psum_tr = psum_pool.tile([128, SC * BPC], F32, tag="tr", bufs=1)
                for i in range(SC):
                    nc.tensor.transpose(psum_tr[:, i * BPC:(i + 1) * BPC],
                                        soft_sb[0:BPC, i * 128:(i + 1) * 128],
                                        identity[0:BPC, 0:BPC])
                soft_pt = pool.tile([128, SC * BPC], mm_dt, tag="softpt", bufs=2)
                nc.scalar.copy(soft_pt, psum_tr)

                # mm3: matched[d] = sum_s soft[s] * A[s, d]
                psum_m = [psum_pool.tile([1, 512], F32, tag="m", name=f"psum_m{j}")
                          for j in range(2)]
                for i in range(SC):
                    for j in range(2):
                        nc.tensor.matmul(
                            psum_m[j],
                            lhsT=soft_pt[:, i * BPC + b:i * BPC + b + 1],
                            rhs=anat_tiles[i][:, j * 512:(j + 1) * 512],
                            start=(i == 0), stop=(i == SC - 1))
                matched_sb = pool.tile([1, D], F32, tag="matched", bufs=2)
                nc.scalar.copy(matched_sb[:, :512], psum_m[0])
                nc.scalar.copy(matched_sb[:, 512:], psum_m[1])
                nc.sync.dma_start(out=matched_d[b:b + 1, :], in_=matched_sb)

    nc.compile()
    return nc


def prep_inputs(att_hidden, rnn_state, W, U):
    rs = np.ascontiguousarray(rnn_state.reshape(B, D), dtype=np.float32)
    at = np.ascontiguousarray(att_hidden.transpose(0, 2, 1), dtype=np.float32)
    an = np.ascontiguousarray(att_hidden, dtype=np.float32)
    wt = np.ascontiguousarray(W.T, dtype=np.float32)
    ut = np.ascontiguousarray(U.T, dtype=np.float32)
    rst = np.ascontiguousarray(rs.T, dtype=np.float32)
    in_maps = []
    for c in range(NCORES):
        sl = slice(c * BPC, (c + 1) * BPC)
        in_maps.append({
            "at": at[sl], "an": an[sl], "wt": wt, "ut": ut,
            "rst": np.ascontiguousarray(rst[:, sl]),
        })
    return in_maps


def assemble(results):
    matched = np.concatenate([results[c]["matched"] for c in range(NCORES)], axis=0)
    soft = np.concatenate([results[c]["soft"] for c in range(NCORES)], axis=0)
    return matched.astype(np.float32), soft.astype(np.float32)


def kernel(att_hidden, rnn_state, W, U):
    if "nc" not in _CACHE:
        _CACHE["nc"] = build_program()
    in_maps = prep_inputs(att_hidden, rnn_state, W, U)
    res = bass_utils.run_bass_kernel_spmd(
        _CACHE["nc"], in_maps, core_ids=list(range(NCORES)))
    return assemble(res.results)


# revision 11
# speedup vs baseline: 1.0786x; 1.0141x over previous
"""AttnMatch Trainium2 kernel: Bahdanau-style additive attention.

  rs       = rnn_state.reshape(B, D)
  att_proj = relu(att_hidden @ W.T + (rs @ U.T)[:, None, :])   # [B,S,D]
  scores   = einsum('bse,be->bs', att_proj, rs)                # [B,S]
  soft     = softmax(scores, axis=1)                           # [B,S]
  matched  = einsum('bsd,bs->bd', att_hidden, soft)            # [B,D]
  returns (matched, soft)

Sharding: data-parallel over batch B=32 across 8 NeuronCores (4 batches/core).
Each core runs the identical program on its own batch slice; no collectives.

Compute dtype: float32r for all TensorE matmuls (full fp32 precision on TRN2
HW -- single fp32_mode=HIGH pass -- at bf16-class streaming rate); fp32
accumulation in PSUM.

Per-core, per-batch plan (S=2048, D=1024):
  mm1  P.T[e,s] += W.T[d,e].T @ A.T[d,s]  (8 K-chunks of 128, N=512 tiles)
  ACT  pt = relu(psum + u[e])             (u[e] = per-partition bias)
  mm2  scores[4,s] += rsT[e,0:4].T @ pt   (rs stationary, N=512)
  softmax on all 4 rows at partition base 0 (row b meaningful)
  PE-transpose soft row -> [128,16] partition-major chunks
  mm3  matched[1,d] += soft_chunk.T @ A_nat[s,d]  (16 K-chunks, N=512)

DMA choreography: W.T/A.T/U.T loads are split per-128-chunk so they spread
across DMA queues; A-natural (mm3) prefetch is interleaved after each s-tile
so it never starves the mm1-critical A.T loads.
"""

import numpy as np

import concourse.bass as bass
import concourse.mybir as mybir
import concourse.tile as tile
from concourse import bacc, bass_utils
from concourse.masks import make_identity

B, S, D = 32, 2048, 1024
NCORES = 8
BPC = B // NCORES          # batches per core
DC = D // 128              # contraction chunks of 128
SI = S // 512              # s-tiles of 512
SC = S // 128              # s-chunks of 128
F32 = mybir.dt.float32

_CACHE = {}


def build_program(mm_dt=mybir.dt.float32r):
    nc = bacc.Bacc("TRN2", target_bir_lowering=False, debug=False,
                   num_devices=NCORES)
    at_d = nc.dram_tensor("at", [BPC, D, S], mm_dt, kind="ExternalInput").ap()
    an_d = nc.dram_tensor("an", [BPC, S, D], mm_dt, kind="ExternalInput").ap()
    wt_d = nc.dram_tensor("wt", [D, D], mm_dt, kind="ExternalInput").ap()
    ut_d = nc.dram_tensor("ut", [D, D], mm_dt, kind="ExternalInput").ap()
    rst_d = nc.dram_tensor("rst", [D, BPC], mm_dt, kind="ExternalInput").ap()
    matched_d = nc.dram_tensor("matched", [BPC, D], F32, kind="ExternalOutput").ap()
    soft_d = nc.dram_tensor("soft", [BPC, S], F32, kind="ExternalOutput").ap()

    Relu = mybir.ActivationFunctionType.Relu
    Exp = mybir.ActivationFunctionType.Exp

    with tile.TileContext(nc) as tc:
        with (
            tc.tile_pool(name="const", bufs=1) as const_pool,
            tc.tile_pool(name="sbuf", bufs=3) as pool,
            tc.tile_pool(name="psum", bufs=2, space="PSUM") as psum_pool,
        ):
            # ---------------- constants ----------------
            identity = const_pool.tile([128, 128], F32, tag="ident")
            make_identity(nc, identity)

            # W.T resident: [d,e] as DC separate tiles so the first matmul
            # group only waits on chunk 0, and the 4MB load spreads over
            # DMA queues
            wt_view = wt_d.rearrange("(c p) e -> p c e", p=128)
            wt_tiles = []
            for dc in range(DC):
                t = const_pool.tile([128, D], mm_dt, tag=f"wt{dc}",
                                    name=f"wt_{dc}")
                nc.sync.dma_start(out=t, in_=wt_view[:, dc, :])
                wt_tiles.append(t)

            # rs.T resident: [d,b] -> sbuf [128, DC, BPC]
            rst_sb = const_pool.tile([128, DC, BPC], mm_dt, tag="rst")
            nc.sync.dma_start(out=rst_sb, in_=rst_d.rearrange("(c p) b -> p c b", p=128))

            uT_sb = const_pool.tile([128, DC, BPC], F32, tag="uT")

            def emit_u_preamble():
                # u[b,e] = U @ rs_b, computed as rsT.T @ U.T -> [BPC, D],
                # then PE-transposed into [e_part, ec, b] bias layout
                ut_tiles = []
                for dc in range(DC):
                    t = pool.tile([128, D], mm_dt, tag="anat", bufs=18)
                    nc.sync.dma_start(
                        out=t, in_=ut_d.rearrange("(c p) e -> p c e", p=128)[:, dc, :])
                    ut_tiles.append(t)
                u_ps = []
                for h in range(2):
                    ps = psum_pool.tile([BPC, 512], F32, tag="sc4",
                                        name=f"u_ps{h}")
                    for dc in range(DC):
                        nc.tensor.matmul(ps, lhsT=rst_sb[:, dc, :],
                                         rhs=ut_tiles[dc][:, h * 512:(h + 1) * 512],
                                         start=(dc == 0), stop=(dc == DC - 1))
                    u_ps.append(ps)
                u_be = const_pool.tile([BPC, D], F32, tag="u_be")
                nc.scalar.copy(u_be[:, :512], u_ps[0])
                nc.scalar.copy(u_be[:, 512:], u_ps[1])
                for ec in range(DC):
                    ps_tr = psum_pool.tile([128, BPC], F32, tag="tr", bufs=1)
                    nc.tensor.transpose(ps_tr, u_be[0:BPC, ec * 128:(ec + 1) * 128],
                                        identity[0:BPC, 0:BPC])
                    nc.scalar.copy(uT_sb[:, ec, :], ps_tr)

            # ---------------- main loop over batches ----------------
            for b in range(BPC):
                # per-batch scores tile; only row b is meaningful (the mm2
                # stationary operand is all 4 batches' rs, so rows != b hold
                # finite-but-unused cross-batch products)
                soft_sb = pool.tile([BPC, S], F32, tag="scores", bufs=2)
                colmax_sb = pool.tile([BPC, SI], F32, tag="colmax", bufs=2)
                anat_tiles = [None] * SC

                at_view = at_d[b].rearrange("(c p) s -> p c s", p=128)
                for si in range(SI):
                    # A.T chunks for this s-tile, one DMA per 128-d-chunk
                    at_tiles = []
                    for dc in range(DC):
                        t = pool.tile([128, 512], mm_dt, tag="at", bufs=24,
                                      name=f"at_{dc}")
                        nc.sync.dma_start(
                            out=t, in_=at_view[:, dc, si * 512:(si + 1) * 512])
                        at_tiles.append(t)

                    psum_sc = psum_pool.tile([BPC, 512], F32, tag="sc4")
                    for e in range(DC):
                        psum_p = psum_pool.tile([128, 512], F32, tag="big", bufs=3)
                        for dc in range(DC):
                            nc.tensor.matmul(
                                psum_p,
                                lhsT=wt_tiles[dc][:, e * 128:(e + 1) * 128],
                                rhs=at_tiles[dc],
                                start=(dc == 0), stop=(dc == DC - 1))
                        pt_tile = pool.tile([128, 512], mm_dt, tag="pt", bufs=4)
                        nc.scalar.activation(pt_tile, psum_p, Relu,
                                             bias=uT_sb[:, e, b:b + 1], scale=1.0)
                        nc.tensor.matmul(psum_sc, lhsT=rst_sb[:, e, :],
                                         rhs=pt_tile,
                                         start=(e == 0), stop=(e == DC - 1))
                    nc.scalar.copy(soft_sb[:, si * 512:(si + 1) * 512], psum_sc)
                    # incremental max per s-tile (off the critical tail path)
                    nc.vector.reduce_max(colmax_sb[:, si:si + 1], psum_sc,
                                         axis=mybir.AxisListType.X)

                    if b == 0 and si == 0:
                        # u preamble emitted after the first s-tile's compute:
                        # its DMAs queue behind the mm1-critical W.T/A.T loads;
                        # the first relu-bias consumer is 3 psum groups in
                        emit_u_preamble()

                    # interleave mm3 prefetch: 4 A-natural chunks per s-tile
                    for i in range(si * 4, si * 4 + 4):
                        t = pool.tile([128, D], mm_dt, tag="anat", bufs=18,
                                      name=f"anat_{i}")
                        nc.sync.dma_start(out=t, in_=an_d[b, i * 128:(i + 1) * 128, :])
                        anat_tiles[i] = t

                # softmax: run on all 4 rows (partition-parallel, same cost);
                # only row b is meaningful. Engine APs must start at
                # partition 0, so never slice rows on compute ops.
                negm_sb = pool.tile([BPC, 1], F32, tag="negm", bufs=2)
                sumexp_sb = pool.tile([BPC, 1], F32, tag="sumexp", bufs=2)
                rz_sb = pool.tile([BPC, 1], F32, tag="rz", bufs=2)
                rz0_sb = pool.tile([1, 1], F32, tag="rz0", bufs=2)
                nc.vector.reduce_max(negm_sb, colmax_sb,
                                     axis=mybir.AxisListType.X, negate=True)
                # unnormalized exp; normalization folds into the matched
                # PSUM->SBUF copy (scale=1/Z) and a deferred output multiply
                nc.scalar.activation(soft_sb, soft_sb,
                                     Exp, bias=negm_sb, scale=1.0,
                                     accum_out=sumexp_sb)
                nc.vector.reciprocal(rz_sb, sumexp_sb)

                # transpose exp row -> [128, SC] partition-major (critical path)
                psum_tr = psum_pool.tile([128, SC * BPC], F32, tag="tr", bufs=1)
                for i in range(SC):
                    nc.tensor.transpose(psum_tr[:, i * BPC:(i + 1) * BPC],
                                        soft_sb[0:BPC, i * 128:(i + 1) * 128],
                                        identity[0:BPC, 0:BPC])
                soft_pt = pool.tile([128, SC * BPC], mm_dt, tag="softpt", bufs=2)
                nc.scalar.copy(soft_pt, psum_tr)

                # normalized soft output (off critical path)
                nc.vector.tensor_scalar_mul(soft_sb, soft_sb, rz_sb)
                nc.sync.dma_start(out=soft_d[b:b + 1, :], in_=soft_sb[b:b + 1, :])
                # 1/Z to partition 0 for the matched scale (cross-partition
                # moves are DMA-only)
                nc.sync.dma_start(out=rz0_sb, in_=rz_sb[b:b + 1, :])

                # mm3: matched[d] = (sum_s exp[s] * A[s, d]) / Z
                psum_m = [psum_pool.tile([1, 512], F32, tag="m", name=f"psum_m{j}")
                          for j in range(2)]
                for i in range(SC):
                    for j in range(2):
                        nc.tensor.matmul(
                            psum_m[j],
                            lhsT=soft_pt[:, i * BPC + b:i * BPC + b + 1],
                            rhs=anat_tiles[i][:, j * 512:(j + 1) * 512],
                            start=(i == 0), stop=(i == SC - 1))
                matched_sb = pool.tile([1, D], F32, tag="matched", bufs=2)
                nc.scalar.mul(matched_sb[:, :512], psum_m[0], rz0_sb)
                nc.scalar.mul(matched_sb[:, 512:], psum_m[1], rz0_sb)
                nc.sync.dma_start(out=matched_d[b:b + 1, :], in_=matched_sb)

    nc.compile()
    return nc


def prep_inputs(att_hidden, rnn_state, W, U):
    rs = np.ascontiguousarray(rnn_state.reshape(B, D), dtype=np.float32)
    at = np.ascontiguousarray(att_hidden.transpose(0, 2, 1), dtype=np.float32)
    an = np.ascontiguousarray(att_hidden, dtype=np.float32)
    wt = np.ascontiguousarray(W.T, dtype=np.float32)
    ut = np.ascontiguousarray(U.T, dtype=np.float32)
    rst = np.ascontiguousarray(rs.T, dtype=np.float32)
    in_maps = []
    for c in range(NCORES):
        sl = slice(c * BPC, (c + 1) * BPC)
        in_maps.append({
            "at": at[sl], "an": an[sl], "wt": wt, "ut": ut,
            "rst": np.ascontiguousarray(rst[:, sl]),
        })
    return in_maps


def assemble(results):
    matched = np.concatenate([results[c]["matched"] for c in range(NCORES)], axis=0)
    soft = np.concatenate([results[c]["soft"] for c in range(NCORES)], axis=0)
    return matched.astype(np.float32), soft.astype(np.float32)


def kernel(att_hidden, rnn_state, W, U):
    if "nc" not in _CACHE:
        _CACHE["nc"] = build_program()
    in_maps = prep_inputs(att_hidden, rnn_state, W, U)
    res = bass_utils.run_bass_kernel_spmd(
        _CACHE["nc"], in_maps, core_ids=list(range(NCORES)))
    return assemble(res.results)
